# revision 3
# baseline (speedup 1.0000x reference)
"""Causal self-attention (B=4, T=2048, C=1024, 16 heads) on 8 trn2 NeuronCores.

Sharding: core c handles batch b = c//2 and head-group hg = c%2 (8 heads each).
Per-core kernel computes, for its 8 heads:
  qkv projections (transposed layouts), causal flash attention, and the
  head-group partial of the output projection (outT = Wp_rows^T @ yT, [C, T]).
Host combines: out[b] = (outT[2b] + outT[2b+1]).T + b_proj.

All matmuls run as float32r (full-rate fp32 on the PE, ~1e-4 rel err).
Softmax runs without max-subtraction (logits are in [-7, 7] for randn inputs;
exp is computed in fp32 by the scalar engine, denominators via an appended
ones-column in the att@v matmul).
"""

import numpy as np
from contextlib import ExitStack

import concourse.bass as bass
from concourse import bacc
import concourse.tile as tile
from concourse import mybir
from concourse.bass_utils import run_bass_kernel_spmd

F32 = mybir.dt.float32
F32R = mybir.dt.float32r
AF = mybir.ActivationFunctionType

B, T, C = 4, 2048, 1024
NH_TOT, D = 16, 64
NHL = 8            # local heads per core
G = 4              # head pairs per core
KT = 8             # c_in k-tiles of 128
TB = 256           # phase-1 token block
NTB = T // TB      # 8
QB = 512           # attention q block
NQ = T // QB       # 4
NKT = T // 128     # 16 token k-tiles
MASK_NEG = -30000.0

_CACHE = {}


def build_nc(debug=False):
    key = ("nc", debug)
    if key in _CACHE:
        return _CACHE[key]
    nc = bacc.Bacc("TRN2", target_bir_lowering=False, debug=False, num_devices=8)

    xT = nc.dram_tensor("xT", [C, T], F32, kind="ExternalInput").ap()
    wq = nc.dram_tensor("wq", [C, 512], F32, kind="ExternalInput").ap()
    wk = nc.dram_tensor("wk", [C, 512], F32, kind="ExternalInput").ap()
    wv = nc.dram_tensor("wv", [C, 512], F32, kind="ExternalInput").ap()
    wp = nc.dram_tensor("wp", [512, C], F32, kind="ExternalInput").ap()
    bq = nc.dram_tensor("bq", [512], F32, kind="ExternalInput").ap()
    bk = nc.dram_tensor("bk", [512], F32, kind="ExternalInput").ap()
    bv = nc.dram_tensor("bv", [512], F32, kind="ExternalInput").ap()
    maskd = nc.dram_tensor("maskd", [128, 128], F32, kind="ExternalInput").ap()
    vonesd = nc.dram_tensor("vonesd", [128, NKT * NHL], F32, kind="ExternalInput").ap()
    outT = nc.dram_tensor("outT", [C, T], F32, kind="ExternalOutput").ap()

    dbg = {}
    if debug:
        dbg["kT"] = nc.dram_tensor("dbg_kT", [128, G, NKT, 128], F32, kind="ExternalOutput").ap()
        dbg["vaug"] = nc.dram_tensor("dbg_vaug", [128, NKT, NHL, 65], F32, kind="ExternalOutput").ap()
        dbg["qT"] = nc.dram_tensor("dbg_qT", [NQ, G, 128, QB], F32, kind="ExternalOutput").ap()
        dbg["yraw"] = nc.dram_tensor("dbg_yraw", [NQ, G, 128, QB], F32, kind="ExternalOutput").ap()
        dbg["sums"] = nc.dram_tensor("dbg_sums", [NQ, NHL, QB], F32, kind="ExternalOutput").ap()

    with tile.TileContext(nc) as tc:
        with ExitStack() as ctx:
            sing = ctx.enter_context(tc.tile_pool(name="sing", bufs=1))
            wts = ctx.enter_context(tc.tile_pool(name="wts", bufs=3))
            wpp = ctx.enter_context(tc.tile_pool(name="wpp", bufs=2))
            xtp = ctx.enter_context(tc.tile_pool(name="xtp", bufs=2))
            qtp = ctx.enter_context(tc.tile_pool(name="qtp", bufs=8))
            axp = ctx.enter_context(tc.tile_pool(name="axp", bufs=3))
            yrp = ctx.enter_context(tc.tile_pool(name="yrp", bufs=4))
            ynp = ctx.enter_context(tc.tile_pool(name="ynp", bufs=4))
            smp = ctx.enter_context(tc.tile_pool(name="smp", bufs=2))
            bcp = ctx.enter_context(tc.tile_pool(name="bcp", bufs=2))
            osp = ctx.enter_context(tc.tile_pool(name="osp", bufs=2))
            drp = ctx.enter_context(tc.tile_pool(name="drp", bufs=2, space="DRAM"))
            ps_y = ctx.enter_context(tc.tile_pool(name="ps_y", bufs=2, space="PSUM"))
            ps_a = ctx.enter_context(tc.tile_pool(name="ps_a", bufs=3, space="PSUM"))
            ps_m = ctx.enter_context(tc.tile_pool(name="ps_m", bufs=3, space="PSUM"))

            # ---- constants / weights ----
            wq_s = wts.tile([128, KT, 512], F32R, tag="w3")
            wk_s = wts.tile([128, KT, 512], F32R, tag="w3")
            wv_s = wts.tile([128, KT, 512], F32R, tag="w3")
            nc.sync.dma_start(out=wq_s, in_=wq.rearrange("(kt p) m -> p kt m", p=128).bitcast(F32R))
            nc.sync.dma_start(out=wk_s, in_=wk.rearrange("(kt p) m -> p kt m", p=128).bitcast(F32R))
            nc.sync.dma_start(out=wv_s, in_=wv.rearrange("(kt p) m -> p kt m", p=128).bitcast(F32R))
            bq_s = sing.tile([128, G], F32)
            bk_s = sing.tile([128, G], F32)
            nc.sync.dma_start(out=bq_s, in_=bq.rearrange("(g p) -> p g", p=128))
            nc.sync.dma_start(out=bk_s, in_=bk.rearrange("(g p) -> p g", p=128))
            bv_s = sing.tile([128, 512], F32)
            nc.sync.dma_start(
                out=bv_s,
                in_=bass.AP(tensor=bv.tensor, offset=bv.offset, ap=[[0, 128]] + list(bv.ap)),
            )
            mask_s = sing.tile([128, 128], F32)
            nc.sync.dma_start(out=mask_s, in_=maskd)

            # persistent K^T and V (augmented with a ones column per head)
            kT_s = sing.tile([128, G, NKT, 128], F32R)
            v_aug = sing.tile([128, NKT, NHL, 65], F32R)
            nc.sync.dma_start(
                out=v_aug[:, :, :, 64:65],
                in_=vonesd.rearrange("p (t h one) -> p t h one", h=NHL, one=1).bitcast(F32R),
            )

            for qi in range(NQ):
                # ---------- phase 1: qkv for token blocks 2qi, 2qi+1 ----------
                for tb in (2 * qi, 2 * qi + 1):
                    xt = xtp.tile([128, KT, TB], F32R)
                    nc.sync.dma_start(
                        out=xt,
                        in_=xT.rearrange("(kt p) (tb tt) -> p kt tb tt", p=128, tt=TB)[:, :, tb, :].bitcast(F32R),
                    )
                    half = tb % 2
                    # q and k: out psum [128(hpair ch), TB]
                    for g in range(G):
                        for which in range(2):  # 0 = q, 1 = k
                            w_s, b_sc = (wq_s, bq_s) if which == 0 else (wk_s, bk_s)
                            pqk = ps_m.tile([128, TB], F32, tag="psm")
                            for kt in range(KT):
                                nc.tensor.matmul(
                                    pqk,
                                    w_s[:, kt, g * 128:(g + 1) * 128],
                                    xt[:, kt, :],
                                    start=(kt == 0),
                                    stop=(kt == KT - 1),
                                )
                            if which == 0:
                                qt = _CACHE.setdefault(("qt", qi, g), None)
                                # allocate the [128, QB] q tile on first half
                                if half == 0:
                                    qt = qtp.tile([128, QB], F32R, tag="qt")
                                    _CACHE[("qt", qi, g)] = qt
                                else:
                                    qt = _CACHE[("qt", qi, g)]
                                nc.vector.tensor_scalar_add(
                                    out=qt[:, half * TB:(half + 1) * TB],
                                    in0=pqk,
                                    scalar1=b_sc[:, g:g + 1],
                                )
                            else:
                                nc.vector.tensor_scalar_add(
                                    out=kT_s[:, g, 2 * tb:2 * tb + 2, :].rearrange("p a b -> p (a b)"),
                                    in0=pqk,
                                    scalar1=b_sc[:, g:g + 1],
                                )
                    # v: out psum [128(tok), 512(h d)]
                    for th in range(2):
                        pv = ps_m.tile([128, 512], F32, tag="psm")
                        for kt in range(KT):
                            nc.tensor.matmul(
                                pv,
                                xt[:, kt, th * 128:(th + 1) * 128],
                                wv_s[:, kt, :],
                                start=(kt == 0),
                                stop=(kt == KT - 1),
                            )
                        tt = 2 * tb + th
                        nc.vector.tensor_add(
                            out=v_aug[:, tt, :, 0:64],
                            in0=pv.rearrange("p (h d) -> p h d", d=64),
                            in1=bv_s.rearrange("p (h d) -> p h d", d=64),
                        )

                # ---------- phase 2: attention for q block qi ----------
                bounce = drp.tile([NHL, QB], F32)
                for h in range(NHL):
                    g, e = h // 2, h % 2
                    base = e * 64
                    qt = _CACHE[("qt", qi, g)]
                    pj = ps_y.tile([65, QB], F32, tag="psy")
                    njt = 4 * qi + 4
                    for j in range(njt):
                        d = j - 4 * qi
                        c0 = 128 * d if d >= 0 else 0
                        pa = ps_a.tile([128, QB], F32, tag="psa")
                        nc.tensor.matmul(
                            pa[:, c0:QB],
                            kT_s[base:base + 64, g, j, :],
                            qt[base:base + 64, c0:QB],
                            start=True,
                            stop=True,
                        )
                        if d >= 0:
                            nc.vector.tensor_add(
                                out=pa[:, c0:c0 + 128],
                                in0=pa[:, c0:c0 + 128],
                                in1=mask_s,
                            )
                        ax = axp.tile([128, QB], F32R, tag="ax")
                        nc.scalar.activation(out=ax[:, c0:QB], in_=pa[:, c0:QB], func=AF.Exp)
                        vsl = v_aug[:, j, h, :]
                        if d < 0:
                            nc.tensor.matmul(pj, vsl, ax, start=(j == 0), stop=False)
                        else:
                            nc.tensor.matmul(
                                pj[:, c0:c0 + 128], vsl, ax[:, c0:c0 + 128],
                                start=(j == 0), stop=True,
                            )
                            if c0 + 128 < QB:
                                nc.tensor.matmul(
                                    pj[:, c0 + 128:QB], vsl, ax[:, c0 + 128:QB],
                                    start=(j == 0), stop=False,
                                )
                    # stash raw y and the softmax denominators
                    if e == 0:
                        yr = yrp.tile([128, QB], F32, tag="yr")
                        _CACHE[("yr", qi, g)] = yr
                    else:
                        yr = _CACHE[("yr", qi, g)]
                    nc.vector.tensor_copy(yr[base:base + 64, :], pj[0:64, :])
                    s1 = smp.tile([1, QB], F32, tag="s1")
                    nc.scalar.activation(out=s1, in_=pj[64:65, :], func=AF.Copy)
                    nc.sync.dma_start(out=bounce[h:h + 1, :], in_=s1)
                    if debug:
                        nc.sync.dma_start(out=dbg["sums"][qi, h, :].rearrange("(one q) -> one q", one=1), in_=s1)

                # ---------- normalization ----------
                rin = smp.tile([NHL, QB], F32, tag="rin")
                nc.sync.dma_start(out=rin, in_=bounce)
                rcp = smp.tile([NHL, QB], F32, tag="rcp")
                nc.vector.reciprocal(rcp, rin)
                bounce2 = drp.tile([NHL, QB], F32, tag="b2")
                nc.sync.dma_start(out=bounce2, in_=rcp)
                for h in range(NHL):
                    g, e = h // 2, h % 2
                    base = e * 64
                    rrow = smp.tile([1, QB], F32, tag="rrow")
                    nc.sync.dma_start(out=rrow, in_=bounce2[h:h + 1, :])
                    bc = bcp.tile([128, QB], F32, tag="bc")
                    nc.gpsimd.partition_broadcast(bc, rrow)
                    if e == 0:
                        yn = ynp.tile([128, QB], F32R, tag="yn")
                        _CACHE[("yn", qi, g)] = yn
                    else:
                        yn = _CACHE[("yn", qi, g)]
                    yr = _CACHE[("yr", qi, g)]
                    nc.vector.tensor_mul(
                        out=yn[base:base + 64, :],
                        in0=yr[base:base + 64, :],
                        in1=bc[base:base + 64, :],
                    )
                if debug:
                    for g in range(G):
                        nc.sync.dma_start(out=dbg["qT"][qi, g], in_=_CACHE[("qt", qi, g)].bitcast(F32))
                        nc.sync.dma_start(out=dbg["yraw"][qi, g], in_=_CACHE[("yr", qi, g)])

                # ---------- phase 3: output projection for q block qi ----------
                for ct in range(C // 128):
                    wp_t = wpp.tile([128, G, 128], F32R, tag="wp")
                    nc.sync.dma_start(
                        out=wp_t,
                        in_=wp.rearrange("(g p) c -> p g c", p=128)[:, :, ct * 128:(ct + 1) * 128].bitcast(F32R),
                    )
                    po = ps_m.tile([128, QB], F32, tag="psm")
                    for g in range(G):
                        nc.tensor.matmul(
                            po,
                            wp_t[:, g, :],
                            _CACHE[("yn", qi, g)],
                            start=(g == 0),
                            stop=(g == G - 1),
                        )
                    ob = osp.tile([128, QB], F32, tag="ob")
                    nc.vector.tensor_copy(ob, po)
                    nc.sync.dma_start(
                        out=outT[ct * 128:(ct + 1) * 128, qi * QB:(qi + 1) * QB],
                        in_=ob,
                    )

            if debug:
                nc.sync.dma_start(out=dbg["kT"], in_=kT_s.bitcast(F32))
                nc.sync.dma_start(out=dbg["vaug"], in_=v_aug.bitcast(F32))

    # drop per-build tile references from the cache
    for k in list(_CACHE):
        if isinstance(k, tuple) and k[0] in ("qt", "yr", "yn"):
            del _CACHE[k]

    nc.compile()
    _CACHE[key] = nc
    return nc


def make_in_maps(x, W_attn, b_attn, W_proj, b_proj):
    mask = np.full((128, 128), MASK_NEG, np.float32)
    kk, qq = np.meshgrid(np.arange(128), np.arange(128), indexing="ij")
    mask[kk <= qq] = 0.0
    vones = np.ones((128, NKT * NHL), np.float32)
    scale = np.float32(1.0 / np.sqrt(D))

    in_maps = []
    for core in range(8):
        b, hg = core // 2, core % 2
        lo, hi = hg * 512, (hg + 1) * 512
        in_maps.append({
            "xT": np.ascontiguousarray(x[b].T),
            "wq": np.ascontiguousarray(W_attn[:, lo:hi]) * scale,
            "wk": np.ascontiguousarray(W_attn[:, C + lo:C + hi]),
            "wv": np.ascontiguousarray(W_attn[:, 2 * C + lo:2 * C + hi]),
            "wp": np.ascontiguousarray(W_proj[lo:hi, :]),
            "bq": b_attn[lo:hi] * scale,
            "bk": np.ascontiguousarray(b_attn[C + lo:C + hi]),
            "bv": np.ascontiguousarray(b_attn[2 * C + lo:2 * C + hi]),
            "maskd": mask,
            "vonesd": vones,
        })
    return in_maps


def kernel(x, W_attn, b_attn, W_proj, b_proj, debug=False, _res_out=None):
    assert x.shape == (B, T, C), x.shape
    nc = build_nc(debug=debug)
    in_maps = make_in_maps(x, W_attn, b_attn, W_proj, b_proj)
    res = run_bass_kernel_spmd(nc, in_maps, core_ids=list(range(8)))
    if _res_out is not None:
        _res_out.extend(res.results)
    out = np.empty((B, T, C), np.float32)
    for b in range(B):
        acc = res.results[2 * b]["outT"] + res.results[2 * b + 1]["outT"]
        out[b] = acc.T + b_proj
    return out


# revision 4
# speedup vs baseline: 41.0088x; 41.0088x over previous
"""Causal self-attention (B=4, T=2048, C=1024, 16 heads) on 8 trn2 NeuronCores.

Sharding: core c handles batch b = c//2 and head-group hg = c%2 (8 heads each).
Per-core kernel computes, for its 8 heads:
  qkv projections (transposed layouts), causal flash attention, and the
  head-group partial of the output projection (outT = Wp_rows^T @ yT, [C, T]).
Host combines: out[b] = (outT[2b] + outT[2b+1]).T + b_proj.

All matmuls run as float32r (full-rate fp32 on the PE, ~1e-4 rel err).
Softmax runs without max-subtraction (logits are in [-7, 7] for randn inputs;
exp is computed in fp32 by the scalar engine, denominators via an appended
ones-column in the att@v matmul).
"""

import numpy as np
from contextlib import ExitStack

import concourse.bass as bass
from concourse import bacc
import concourse.tile as tile
from concourse import mybir
from concourse.bass_utils import run_bass_kernel_spmd

F32 = mybir.dt.float32
F32R = mybir.dt.float32r
AF = mybir.ActivationFunctionType

B, T, C = 4, 2048, 1024
NH_TOT, D = 16, 64
NHL = 8            # local heads per core
G = 4              # head pairs per core
KT = 8             # c_in k-tiles of 128
TB = 256           # phase-1 token block
NTB = T // TB      # 8
QB = 512           # attention q block
NQ = T // QB       # 4
NKT = T // 128     # 16 token k-tiles
MASK_NEG = -30000.0

_CACHE = {}


def build_nc(debug=False, reps=1):
    key = ("nc", debug, reps)
    if key in _CACHE:
        return _CACHE[key]
    nc = bacc.Bacc("TRN2", target_bir_lowering=False, debug=False, num_devices=8)

    xT = nc.dram_tensor("xT", [C, T], F32, kind="ExternalInput").ap()
    wq = nc.dram_tensor("wq", [C, 512], F32, kind="ExternalInput").ap()
    wk = nc.dram_tensor("wk", [C, 512], F32, kind="ExternalInput").ap()
    wv = nc.dram_tensor("wv", [C, 512], F32, kind="ExternalInput").ap()
    wp = nc.dram_tensor("wp", [512, C], F32, kind="ExternalInput").ap()
    bq = nc.dram_tensor("bq", [512], F32, kind="ExternalInput").ap()
    bk = nc.dram_tensor("bk", [512], F32, kind="ExternalInput").ap()
    bv = nc.dram_tensor("bv", [512], F32, kind="ExternalInput").ap()
    maskd = nc.dram_tensor("maskd", [128, 128], F32, kind="ExternalInput").ap()
    vonesd = nc.dram_tensor("vonesd", [128, NKT * NHL], F32, kind="ExternalInput").ap()
    outT = nc.dram_tensor("outT", [C, T], F32, kind="ExternalOutput").ap()

    dbg = {}
    if debug:
        dbg["kT"] = nc.dram_tensor("dbg_kT", [128, G, NKT, 128], F32, kind="ExternalOutput").ap()
        dbg["vaug"] = nc.dram_tensor("dbg_vaug", [128, NKT, NHL, 65], F32, kind="ExternalOutput").ap()
        dbg["qT"] = nc.dram_tensor("dbg_qT", [NQ, G, 128, QB], F32, kind="ExternalOutput").ap()
        dbg["yraw"] = nc.dram_tensor("dbg_yraw", [NQ, G, 128, QB], F32, kind="ExternalOutput").ap()
        dbg["sums"] = nc.dram_tensor("dbg_sums", [NQ, NHL, QB], F32, kind="ExternalOutput").ap()

    with tile.TileContext(nc) as tc:
        with ExitStack() as ctx:
            sing = ctx.enter_context(tc.tile_pool(name="sing", bufs=1))
            wts = ctx.enter_context(tc.tile_pool(name="wts", bufs=3))
            wpp = ctx.enter_context(tc.tile_pool(name="wpp", bufs=2))
            xtp = ctx.enter_context(tc.tile_pool(name="xtp", bufs=2))
            qtp = ctx.enter_context(tc.tile_pool(name="qtp", bufs=8))
            axp = ctx.enter_context(tc.tile_pool(name="axp", bufs=3))
            yrp = ctx.enter_context(tc.tile_pool(name="yrp", bufs=4))
            ynp = ctx.enter_context(tc.tile_pool(name="ynp", bufs=4))
            smp = ctx.enter_context(tc.tile_pool(name="smp", bufs=2))
            bcp = ctx.enter_context(tc.tile_pool(name="bcp", bufs=2))
            osp = ctx.enter_context(tc.tile_pool(name="osp", bufs=2))
            drp = ctx.enter_context(tc.tile_pool(name="drp", bufs=2, space="DRAM"))
            ps_y = ctx.enter_context(tc.tile_pool(name="ps_y", bufs=2, space="PSUM"))
            ps_a = ctx.enter_context(tc.tile_pool(name="ps_a", bufs=3, space="PSUM"))
            ps_m = ctx.enter_context(tc.tile_pool(name="ps_m", bufs=3, space="PSUM"))

            # ---- constants / weights ----
            wq_s = wts.tile([128, KT, 512], F32R, tag="w3")
            wk_s = wts.tile([128, KT, 512], F32R, tag="w3")
            wv_s = wts.tile([128, KT, 512], F32R, tag="w3")
            nc.sync.dma_start(out=wq_s, in_=wq.rearrange("(kt p) m -> p kt m", p=128).bitcast(F32R))
            nc.sync.dma_start(out=wk_s, in_=wk.rearrange("(kt p) m -> p kt m", p=128).bitcast(F32R))
            nc.sync.dma_start(out=wv_s, in_=wv.rearrange("(kt p) m -> p kt m", p=128).bitcast(F32R))
            bq_s = sing.tile([128, G], F32)
            bk_s = sing.tile([128, G], F32)
            nc.sync.dma_start(out=bq_s, in_=bq.rearrange("(g p) -> p g", p=128))
            nc.sync.dma_start(out=bk_s, in_=bk.rearrange("(g p) -> p g", p=128))
            bv_s = sing.tile([128, 512], F32)
            nc.sync.dma_start(
                out=bv_s,
                in_=bass.AP(tensor=bv.tensor, offset=bv.offset, ap=[[0, 128]] + list(bv.ap)),
            )
            mask_s = sing.tile([128, 128], F32)
            nc.sync.dma_start(out=mask_s, in_=maskd)

            # persistent K^T and V (augmented with a ones column per head)
            kT_s = sing.tile([128, G, NKT, 128], F32R)
            v_aug = sing.tile([128, NKT, NHL, 65], F32R)
            nc.sync.dma_start(
                out=v_aug[:, :, :, 64:65],
                in_=vonesd.rearrange("p (t h one) -> p t h one", h=NHL, one=1).bitcast(F32R),
            )

            for rep_qi in range(reps * NQ):
                qi = rep_qi % NQ
                # ---------- phase 1: qkv for token blocks 2qi, 2qi+1 ----------
                for tb in (2 * qi, 2 * qi + 1):
                    xt = xtp.tile([128, KT, TB], F32R)
                    nc.sync.dma_start(
                        out=xt,
                        in_=xT.rearrange("(kt p) (tb tt) -> p kt tb tt", p=128, tt=TB)[:, :, tb, :].bitcast(F32R),
                    )
                    half = tb % 2
                    # q and k: out psum [128(hpair ch), TB]
                    for g in range(G):
                        for which in range(2):  # 0 = q, 1 = k
                            w_s, b_sc = (wq_s, bq_s) if which == 0 else (wk_s, bk_s)
                            pqk = ps_m.tile([128, TB], F32, tag="psm")
                            for kt in range(KT):
                                nc.tensor.matmul(
                                    pqk,
                                    w_s[:, kt, g * 128:(g + 1) * 128],
                                    xt[:, kt, :],
                                    start=(kt == 0),
                                    stop=(kt == KT - 1),
                                )
                            if which == 0:
                                qt = _CACHE.setdefault(("qt", qi, g), None)
                                # allocate the [128, QB] q tile on first half
                                if half == 0:
                                    qt = qtp.tile([128, QB], F32R, tag="qt")
                                    _CACHE[("qt", qi, g)] = qt
                                else:
                                    qt = _CACHE[("qt", qi, g)]
                                nc.vector.tensor_scalar_add(
                                    out=qt[:, half * TB:(half + 1) * TB],
                                    in0=pqk,
                                    scalar1=b_sc[:, g:g + 1],
                                )
                            else:
                                nc.vector.tensor_scalar_add(
                                    out=kT_s[:, g, 2 * tb:2 * tb + 2, :].rearrange("p a b -> p (a b)"),
                                    in0=pqk,
                                    scalar1=b_sc[:, g:g + 1],
                                )
                    # v: out psum [128(tok), 512(h d)]
                    for th in range(2):
                        pv = ps_m.tile([128, 512], F32, tag="psm")
                        for kt in range(KT):
                            nc.tensor.matmul(
                                pv,
                                xt[:, kt, th * 128:(th + 1) * 128],
                                wv_s[:, kt, :],
                                start=(kt == 0),
                                stop=(kt == KT - 1),
                            )
                        tt = 2 * tb + th
                        nc.vector.tensor_add(
                            out=v_aug[:, tt, :, 0:64],
                            in0=pv.rearrange("p (h d) -> p h d", d=64),
                            in1=bv_s.rearrange("p (h d) -> p h d", d=64),
                        )

                # ---------- phase 2: attention for q block qi ----------
                bounce = drp.tile([NHL, QB], F32)
                for h in range(NHL):
                    g, e = h // 2, h % 2
                    base = e * 64
                    qt = _CACHE[("qt", qi, g)]
                    pj = ps_y.tile([65, QB], F32, tag="psy")
                    njt = 4 * qi + 4
                    for j in range(njt):
                        d = j - 4 * qi
                        c0 = 128 * d if d >= 0 else 0
                        pa = ps_a.tile([128, QB], F32, tag="psa")
                        nc.tensor.matmul(
                            pa[:, c0:QB],
                            kT_s[base:base + 64, g, j, :],
                            qt[base:base + 64, c0:QB],
                            start=True,
                            stop=True,
                        )
                        if d >= 0:
                            nc.vector.tensor_add(
                                out=pa[:, c0:c0 + 128],
                                in0=pa[:, c0:c0 + 128],
                                in1=mask_s,
                            )
                        ax = axp.tile([128, QB], F32R, tag="ax")
                        nc.scalar.activation(out=ax[:, c0:QB], in_=pa[:, c0:QB], func=AF.Exp)
                        vsl = v_aug[:, j, h, :]
                        if d < 0:
                            nc.tensor.matmul(pj, vsl, ax, start=(j == 0), stop=False)
                        else:
                            nc.tensor.matmul(
                                pj[:, c0:c0 + 128], vsl, ax[:, c0:c0 + 128],
                                start=(j == 0), stop=True,
                            )
                            if c0 + 128 < QB:
                                nc.tensor.matmul(
                                    pj[:, c0 + 128:QB], vsl, ax[:, c0 + 128:QB],
                                    start=(j == 0), stop=False,
                                )
                    # stash raw y and the softmax denominators
                    if e == 0:
                        yr = yrp.tile([128, QB], F32, tag="yr")
                        _CACHE[("yr", qi, g)] = yr
                    else:
                        yr = _CACHE[("yr", qi, g)]
                    nc.vector.tensor_copy(yr[base:base + 64, :], pj[0:64, :])
                    s1 = smp.tile([1, QB], F32, tag="s1")
                    nc.scalar.activation(out=s1, in_=pj[64:65, :], func=AF.Copy)
                    nc.sync.dma_start(out=bounce[h:h + 1, :], in_=s1)
                    if debug:
                        nc.sync.dma_start(out=dbg["sums"][qi, h, :].rearrange("(one q) -> one q", one=1), in_=s1)

                # ---------- normalization ----------
                rin = smp.tile([NHL, QB], F32, tag="rin")
                nc.sync.dma_start(out=rin, in_=bounce)
                rcp = smp.tile([NHL, QB], F32, tag="rcp")
                nc.vector.reciprocal(rcp, rin)
                bounce2 = drp.tile([NHL, QB], F32, tag="b2")
                nc.sync.dma_start(out=bounce2, in_=rcp)
                for h in range(NHL):
                    g, e = h // 2, h % 2
                    base = e * 64
                    rrow = smp.tile([1, QB], F32, tag="rrow")
                    nc.sync.dma_start(out=rrow, in_=bounce2[h:h + 1, :])
                    bc = bcp.tile([128, QB], F32, tag="bc")
                    nc.gpsimd.partition_broadcast(bc, rrow)
                    if e == 0:
                        yn = ynp.tile([128, QB], F32R, tag="yn")
                        _CACHE[("yn", qi, g)] = yn
                    else:
                        yn = _CACHE[("yn", qi, g)]
                    yr = _CACHE[("yr", qi, g)]
                    nc.vector.tensor_mul(
                        out=yn[base:base + 64, :],
                        in0=yr[base:base + 64, :],
                        in1=bc[base:base + 64, :],
                    )
                if debug:
                    for g in range(G):
                        nc.sync.dma_start(out=dbg["qT"][qi, g], in_=_CACHE[("qt", qi, g)].bitcast(F32))
                        nc.sync.dma_start(out=dbg["yraw"][qi, g], in_=_CACHE[("yr", qi, g)])

                # ---------- phase 3: output projection for q block qi ----------
                for ct in range(C // 128):
                    wp_t = wpp.tile([128, G, 128], F32R, tag="wp")
                    nc.sync.dma_start(
                        out=wp_t,
                        in_=wp.rearrange("(g p) c -> p g c", p=128)[:, :, ct * 128:(ct + 1) * 128].bitcast(F32R),
                    )
                    po = ps_m.tile([128, QB], F32, tag="psm")
                    for g in range(G):
                        nc.tensor.matmul(
                            po,
                            wp_t[:, g, :],
                            _CACHE[("yn", qi, g)],
                            start=(g == 0),
                            stop=(g == G - 1),
                        )
                    ob = osp.tile([128, QB], F32, tag="ob")
                    nc.vector.tensor_copy(ob, po)
                    nc.sync.dma_start(
                        out=outT[ct * 128:(ct + 1) * 128, qi * QB:(qi + 1) * QB],
                        in_=ob,
                    )

            if debug:
                nc.sync.dma_start(out=dbg["kT"], in_=kT_s.bitcast(F32))
                nc.sync.dma_start(out=dbg["vaug"], in_=v_aug.bitcast(F32))

    # drop per-build tile references from the cache
    for k in list(_CACHE):
        if isinstance(k, tuple) and k[0] in ("qt", "yr", "yn"):
            del _CACHE[k]

    nc.compile()
    _CACHE[key] = nc
    return nc


def make_in_maps(x, W_attn, b_attn, W_proj, b_proj):
    mask = np.full((128, 128), MASK_NEG, np.float32)
    kk, qq = np.meshgrid(np.arange(128), np.arange(128), indexing="ij")
    mask[kk <= qq] = 0.0
    vones = np.ones((128, NKT * NHL), np.float32)
    scale = np.float32(1.0 / np.sqrt(D))

    in_maps = []
    for core in range(8):
        b, hg = core // 2, core % 2
        lo, hi = hg * 512, (hg + 1) * 512
        in_maps.append({
            "xT": np.ascontiguousarray(x[b].T),
            "wq": np.ascontiguousarray(W_attn[:, lo:hi]) * scale,
            "wk": np.ascontiguousarray(W_attn[:, C + lo:C + hi]),
            "wv": np.ascontiguousarray(W_attn[:, 2 * C + lo:2 * C + hi]),
            "wp": np.ascontiguousarray(W_proj[lo:hi, :]),
            "bq": b_attn[lo:hi] * scale,
            "bk": np.ascontiguousarray(b_attn[C + lo:C + hi]),
            "bv": np.ascontiguousarray(b_attn[2 * C + lo:2 * C + hi]),
            "maskd": mask,
            "vonesd": vones,
        })
    return in_maps


def kernel(x, W_attn, b_attn, W_proj, b_proj, debug=False, _res_out=None):
    assert x.shape == (B, T, C), x.shape
    nc = build_nc(debug=debug)
    in_maps = make_in_maps(x, W_attn, b_attn, W_proj, b_proj)
    res = run_bass_kernel_spmd(nc, in_maps, core_ids=list(range(8)))
    if _res_out is not None:
        _res_out.extend(res.results)
    out = np.empty((B, T, C), np.float32)
    for b in range(B):
        acc = res.results[2 * b]["outT"] + res.results[2 * b + 1]["outT"]
        out[b] = acc.T + b_proj
    return out
